# revision 21
# baseline (speedup 1.0000x reference)
"""MoE routing kernel for Trainium2 (8 NeuronCores, SPMD data-parallel).

Reference computation (per token t of 16384, D=1024, E=8 experts, top-2, H=128, C=1024):
  scores = softmax(x @ gate_w + gate_b)            [B, E]
  top2 vals/idx; dense expert MLPs h=relu(x@w1+b1); y=h@w2+b2
  out[t] = sum_k val_k * y[idx_k, t]; gates_sum[k] = sum_t val_k  -> [2, C] broadcast

Strategy: shard tokens 8-way (2048/core), replicate weights.
Per core, instead of gather/scatter routing we compute dense expert MLPs but
fold the top-2 selection into a per-(expert,token) weight g[e,t] (softmax value
if selected else 0), so:
  out = sum_e (relu(x@w1_e+b1_e) * g_e) @ w2_e + (g @ b2)
Layouts: x arrives pre-transposed (xT [D, T]) so D sits on partitions for both
the gating matmul and mm1. h is produced as h.T [H, T] (psum), relu+b1 fused in
the ACT eviction, the g row is replicated across partitions with a K=8
one-hot-selector matmul (float32r), and mm2 contracts H with lhsT=hg.T tiles
producing out [T, C] directly; the b2 term rides the same psum accumulation as
a zero-padded K=128 matmul. Gating is exact fp32 (top-2 selection must match
the reference); mm1/mm2 run in bf16 with f32 psum accumulation.
"""

import sys
from contextlib import ExitStack

import numpy as np

sys.path.insert(0, "/opt/trn_rl_repo")

import concourse.bass as bass
import concourse.bacc as bacc
import concourse.mybir as mybir
from concourse.bass_utils import run_bass_kernel_spmd
from concourse.masks import make_identity
from concourse.tile import TileContext

NCORES = 8
B = 16384
TPC = B // NCORES  # tokens per core
D = 1024
E = 8
H = 128
C = 1024
KD = D // 128  # k-chunks of the D contraction
TT = TPC // 128  # token tiles per core
NT = TPC // 512  # 512-wide n-chunks of the token dim
NC2 = C // 512  # 512-wide chunks of the class dim

F32 = mybir.dt.float32
F32R = mybir.dt.float32r
BF16 = mybir.dt.bfloat16
AF = mybir.ActivationFunctionType
ALU = mybir.AluOpType

_CACHE = {}


def _r(ap):
    return ap.bitcast(F32R)


def build_program():
    nc = bacc.Bacc("TRN2", target_bir_lowering=False)

    xT_d = nc.declare_dram_parameter("xT", [D, TPC], F32, isOutput=False)
    w1_d = nc.declare_dram_parameter("w1", [E, D, H], F32, isOutput=False)
    w2_d = nc.declare_dram_parameter("w2", [E, H, C], F32, isOutput=False)
    gw_d = nc.declare_dram_parameter("gw", [D, E], F32, isOutput=False)
    gb_d = nc.declare_dram_parameter("gb", [E], F32, isOutput=False)
    b1_d = nc.declare_dram_parameter("b1", [E, H], F32, isOutput=False)
    b2_d = nc.declare_dram_parameter("b2", [E, C], F32, isOutput=False)
    out_d = nc.declare_dram_parameter("out", [TPC, C], F32, isOutput=True)
    gs_d = nc.declare_dram_parameter("gs", [2, 1], F32, isOutput=True)

    with TileContext(nc) as tc, ExitStack() as ctx:
        pk = ctx.enter_context(tc.tile_pool(name="pk", bufs=1))

        xh = pk.tile([128, KD, TPC], BF16)  # x.T cast to bf16
        w1b = pk.tile([128, KD, E, H], BF16)
        gw_sb = pk.tile([128, KD, E], F32)
        gbr = pk.tile([128, E], F32)
        b1s = pk.tile([128, E], F32)  # b1.T: [h, e]
        b2f = pk.tile([E, C], F32)
        ident = pk.tile([128, 128], F32)
        self_build = pk.tile([8, E, 128], F32)  # staging for selector
        sel = pk.tile([8, E, 128], F32R)  # sel[:, e] = one-hot row e -> replicator
        ones128 = pk.tile([128, 1], F32)
        gT = pk.tile([8, TPC], F32R)  # selected-expert weights, transposed
        gTb = pk.tile([128, TPC], BF16)  # zero-padded bf16 gT for b2 matmul
        b2b = pk.tile([128, C], BF16)  # zero-padded bf16 b2
        v12 = pk.tile([128, TT, 2], F32)  # per-token top-2 softmax values
        gs_sb = pk.tile([2, 1], F32)

        make_identity(nc, ident)
        # sel[p, e, i] = 1 iff p == e  (iota = p - e, fill where == 0)
        nc.gpsimd.memset(self_build, 0.0)
        nc.gpsimd.affine_select(
            out=self_build,
            in_=self_build,
            compare_op=ALU.not_equal,
            fill=1.0,
            base=0,
            pattern=[[-1, E], [0, 128]],
            channel_multiplier=1,
        )
        nc.vector.tensor_copy(sel, self_build)
        nc.vector.memset(gTb, 0.0)
        nc.vector.memset(b2b, 0.0)
        nc.vector.memset(ones128, 1.0)

        nc.sync.dma_start(gw_sb, gw_d.rearrange("(k p) e -> p k e", p=128))
        nc.sync.dma_start(gbr, gb_d[None, :].to_broadcast([128, E]))
        nc.sync.dma_start(b1s, b1_d.rearrange("e h -> h e"))
        nc.sync.dma_start(b2f, b2_d[:, :])
        nc.vector.tensor_copy(b2b[0:E, :], b2f)

        # ---- load x.T (fp32, gating) + cast to bf16 (mm1); load/cast w1 ----
        pA = tc.tile_pool(name="pA", bufs=1)  # region reused later for w2/out
        pB = tc.tile_pool(name="pB", bufs=1)  # region reused later for hg.T
        with pA as pa, pB as pb:
            xTf = pa.tile([128, KD, TPC], F32)
            xT_r = xT_d.rearrange("(k p) t -> p k t", p=128)
            for q in range(8):
                qs = slice(q * (TPC // 8), (q + 1) * (TPC // 8))
                nc.sync.dma_start(xTf[:, :, qs], xT_r[:, :, qs])
            w1f = pb.tile([128, KD, E, H], F32)
            for k in range(KD):
                nc.sync.dma_start(
                    w1f[:, k],
                    w1_d[:, k * 128 : (k + 1) * 128, :].rearrange("e p h -> p e h"),
                )
            for q in range(4):
                qs = slice(q * (TPC // 4), (q + 1) * (TPC // 4))
                for k in range(KD):
                    eng = nc.scalar if k % 2 == 0 else nc.vector
                    if eng is nc.scalar:
                        eng.copy(xh[:, k, qs], xTf[:, k, qs])
                    else:
                        eng.tensor_copy(xh[:, k, qs], xTf[:, k, qs])
            for k in range(KD):
                eng = nc.scalar if k % 2 == 1 else nc.vector
                src = w1f[:, k].rearrange("p e h -> p (e h)")
                dst = w1b[:, k].rearrange("p e h -> p (e h)")
                if eng is nc.scalar:
                    eng.copy(dst, src)
                else:
                    eng.tensor_copy(dst, src)

            # ---- gating: exact fp32 logits in [T, 8] layout ----
            with (
                tc.tile_pool(name="pgs", bufs=2) as pgs,
                tc.tile_pool(name="ppsc", bufs=2, space="PSUM") as ppsc,
                tc.tile_pool(name="ppt", bufs=2, space="PSUM") as ppt,
                tc.tile_pool(name="ppgs", bufs=1, space="PSUM") as ppgs,
            ):
                psum_gs = ppgs.tile([2, 1], F32)
                for tt in range(TT):
                    ts = slice(tt * 128, (tt + 1) * 128)
                    psum_s = ppsc.tile([128, E], F32)
                    for k in range(KD):
                        nc.tensor.matmul(
                            psum_s,
                            lhsT=xTf[:, k, ts],
                            rhs=gw_sb[:, k],
                            start=(k == 0),
                            stop=(k == KD - 1),
                        )
                    scores = pgs.tile([128, E], F32, tag="scores")
                    nc.vector.tensor_tensor(scores, psum_s, gbr, ALU.add)
                    max8 = pgs.tile([128, E], F32, tag="max8")
                    nc.vector.max(out=max8, in_=scores)
                    nm1 = pgs.tile([128, 1], F32, tag="nm1")
                    nc.vector.tensor_scalar_mul(nm1, max8[:, 0:1], -1.0)
                    et = pgs.tile([128, E], F32, tag="et")
                    Z = pgs.tile([128, 1], F32, tag="Z")
                    nc.scalar.activation(et, scores, AF.Exp, bias=nm1, accum_out=Z)
                    # v1 = 1/Z ; v2 = exp(m2 - m1)/Z
                    nc.vector.reciprocal(v12[:, tt, 0:1], Z)
                    v2u = pgs.tile([128, 1], F32, tag="v2u")
                    nc.scalar.activation(v2u, max8[:, 1:2], AF.Exp, bias=nm1)
                    nc.vector.tensor_tensor(
                        v12[:, tt, 1:2], v2u, v12[:, tt, 0:1], ALU.mult
                    )
                    # g[t, e] = softmax value where score >= 2nd max, else 0
                    ge = pgs.tile([128, E], F32, tag="ge")
                    nc.vector.tensor_scalar(
                        ge, scores, max8[:, 1:2], None, op0=ALU.is_ge
                    )
                    g8 = pgs.tile([128, E], F32, tag="g8")
                    nc.vector.tensor_scalar_mul(g8, et, v12[:, tt, 0:1])
                    nc.vector.tensor_tensor(g8, g8, ge, ALU.mult)
                    # transpose to gT[8, T] via PE
                    psum_t = ppt.tile([8, 128], F32)
                    nc.tensor.transpose(psum_t, g8, ident)
                    nc.vector.tensor_copy(gT[:, ts], psum_t)
                    # gates_sum partials: [2,1] += v12[:,tt,:].T @ ones
                    nc.tensor.matmul(
                        psum_gs,
                        lhsT=v12[:, tt, :],
                        rhs=ones128,
                        start=(tt == 0),
                        stop=(tt == TT - 1),
                    )
                nc.vector.tensor_copy(gTb[0:E, :], gT.bitcast(F32))
                nc.vector.tensor_copy(gs_sb, psum_gs)
                nc.sync.dma_start(gs_d[:, :], gs_sb)

        # ---- mm1 (bf16): h.T[e] = relu(w1_e.T @ x.T + b1), scaled by g ----
        pA2 = tc.tile_pool(name="pA2", bufs=1)
        pB2 = tc.tile_pool(name="pB2", bufs=1)
        with pA2 as pa2, pB2 as pb2, tc.tile_pool(name="ph", bufs=2) as ph:
            w2f = pa2.tile([128, E, C], F32)
            w2b = pa2.tile([128, E, C], BF16)
            nc.sync.dma_start(w2f, w2_d.rearrange("e h c -> h e c"))
            for e in range(E):
                eng = nc.scalar if e % 2 == 0 else nc.vector
                if eng is nc.scalar:
                    eng.copy(w2b[:, e], w2f[:, e])
                else:
                    eng.tensor_copy(w2b[:, e], w2f[:, e])
            hgT = pb2.tile([128, E, TPC], BF16)

            with (
                tc.tile_pool(name="pph", bufs=4, space="PSUM") as pph,
                tc.tile_pool(name="ppg", bufs=2, space="PSUM") as ppg,
            ):
                for e in range(E):
                    hT = ph.tile([128, TPC], F32, tag="hT")
                    for n in range(NT):
                        ns = slice(n * 512, (n + 1) * 512)
                        psum_hn = pph.tile([128, 512], F32, tag="psum_h")
                        for k in range(KD):
                            nc.tensor.matmul(
                                psum_hn,
                                lhsT=w1b[:, k, e],
                                rhs=xh[:, k, ns],
                                start=(k == 0),
                                stop=(k == KD - 1),
                            )
                        nc.scalar.activation(
                            hT[:, ns], psum_hn, AF.Relu, bias=b1s[:, e : e + 1]
                        )
                        psum_g = ppg.tile([128, 512], F32)
                        nc.tensor.matmul(
                            psum_g,
                            lhsT=sel[:, e, :],
                            rhs=gT[:, ns],
                            start=True,
                            stop=True,
                        )
                        nc.vector.tensor_tensor(
                            hgT[:, e, ns], hT[:, ns], psum_g, ALU.mult
                        )

            # ---- mm2 (f32r): out[T, C] = sum_e hg.T_e.T @ w2_e + g @ b2 ----
            with (
                tc.tile_pool(name="po", bufs=3) as po,
                tc.tile_pool(name="ppo", bufs=3, space="PSUM") as ppo,
            ):
                for tt in range(TT):
                    ts = slice(tt * 128, (tt + 1) * 128)
                    out_sb = po.tile([128, C], F32, tag="out_sb")
                    for c in range(NC2):
                        cs = slice(c * 512, (c + 1) * 512)
                        psum_oc = ppo.tile([128, 512], F32, tag="psum_o")
                        for e in range(E):
                            nc.tensor.matmul(
                                psum_oc,
                                lhsT=hgT[:, e, ts],
                                rhs=w2b[:, e, cs],
                                start=(e == 0),
                                stop=False,
                            )
                        nc.tensor.matmul(
                            psum_oc,
                            lhsT=gTb[:, ts],
                            rhs=b2b[:, cs],
                            start=False,
                            stop=True,
                        )
                        nc.vector.tensor_copy(out_sb[:, cs], psum_oc)
                    nc.sync.dma_start(out_d[ts, :], out_sb)

    nc.compile()
    return nc


def kernel(**inputs):
    x = np.asarray(inputs["x"], dtype=np.float32)
    gate_w = np.ascontiguousarray(np.asarray(inputs["gate_w"], dtype=np.float32))
    gate_b = np.ascontiguousarray(np.asarray(inputs["gate_b"], dtype=np.float32))
    w1 = np.ascontiguousarray(np.asarray(inputs["w1"], dtype=np.float32))
    b1 = np.ascontiguousarray(np.asarray(inputs["b1"], dtype=np.float32))
    w2 = np.ascontiguousarray(np.asarray(inputs["w2"], dtype=np.float32))
    b2 = np.ascontiguousarray(np.asarray(inputs["b2"], dtype=np.float32))

    if "nc" not in _CACHE:
        _CACHE["nc"] = build_program()
    nc = _CACHE["nc"]

    in_maps = []
    for i in range(NCORES):
        shard = x[i * TPC : (i + 1) * TPC]
        in_maps.append(
            {
                "xT": np.ascontiguousarray(shard.T),
                "w1": w1,
                "w2": w2,
                "gw": gate_w,
                "gb": gate_b,
                "b1": b1,
                "b2": b2,
            }
        )

    res = run_bass_kernel_spmd(nc, in_maps, list(range(NCORES)))
    outs = [np.asarray(res.results[i]["out"]) for i in range(NCORES)]
    gss = [np.asarray(res.results[i]["gs"]) for i in range(NCORES)]
    output = np.concatenate(outs, axis=0).astype(np.float32)
    gs = np.sum(np.stack(gss), axis=0).reshape(2, 1).astype(np.float32)
    gates_sum = np.broadcast_to(gs, (2, C)).copy()
    return (output, gates_sum)
